# revision 42
# baseline (speedup 1.0000x reference)
"""Trainium2 Bass kernel for nn_Block_33105607917680 (gnn_message_passing).

Sharding: batch (2) x N-shard (4) over 8 cores; each core owns 2048 points
of one batch. Per LFP layer, cores compute their h-shard (x @ W, row-major)
and AllGather it into a per-batch-group [8192, 256] bf16 HBM table; KNN
neighbor features are fetched with dma_gather. Gaussian kernel weights are
computed on-device from a host-precomputed rank-7 geometric basis
(pn, pn^2, 1) via TensorE + Exp on ScalarE.

Engine balance: the Activation engine was the measured bottleneck in the
original layout (79% busy), so per-tile work is spread so every engine
stays under the 2.91us/tile gather-DMA floor:
 - the k-reduction "select" is 16 disjoint 8-column matmuls per channel
   half against a shared [128,8] 1/K selection block (rows p of a slot
   feed point p//16 exclusively), nearly free on PE vs the old
   accumulating 128-column form;
 - one fused ACT copy drains both channel halves of the agg PSUM;
 - BN channel-sums ride a 4x-mode DVE tensor_scalar+accum_out pass,
   sumsqs a DVE scalar_tensor_tensor (per tile, in the loop's shadow);
 - BN apply is two DVE passes (tensor_scalar 4x affine + bf16 2x add),
   zero ACT work;
 - BN rsqrt is a DVE-only bit-trick: y0 = bits(0x5EF759DF - (bits(w)>>1))
   with w = (var+eps)/2, one Newton step y*(1.5 - w*y^2) -> rsqrt(var+eps)
   to ~1.7e-3. No Sqrt/Ln table function: the only activation-table sets
   used are Gelu (MLP) and Exp (LFP), 5 loads total, all hoisted off the
   BN tails (dummy ops anchored on late-produced tiles pin their queue
   position against scheduler reordering).
All matmuls are bf16 (fp8/DoubleRow variants were tried and measured:
h2-only fp8 with hi/lo-split w2 leaves a 0.13 max-abs error from the
gelu-output e4m3 cast alone, blowing the 2e-2 budget; accum_out-less
tensor_scalar also fails walrus's TensorScalarPtrReduce verifier, so any
accum_out pass must carry both ALU ops). The residual state is bf16
end-to-end and ships out as a bf16 xout the host upcasts.
BatchNorm batch statistics are AllReduced (sum/sumsq) across all 8 cores.

Channels are relabeled host-side (c=4g+c4 -> 64*c4+g) so the per-group
gaussian weight broadcast becomes a stride-1 read; all weight matrices are
permuted to match and the output is unpermuted on the host.
"""
import sys
sys.path.insert(0, '/opt/trn_rl_repo')

import numpy as np
import ml_dtypes

BF = ml_dtypes.bfloat16
F8 = ml_dtypes.float8_e4m3
B, N, K, DIM, DEPTH, HID = 2, 8192, 16, 256, 4, 1024
D4 = DIM // 4
EPS = 1e-5
NCORES, SHARD = 8, 2048
NT = SHARD // 128            # point tiles per core
ROWS_T = 128 * K             # gathered rows per tile
NSLOT = ROWS_T // 128        # row slots per tile
NCH = 4                      # MLP n-chunks
CHN = SHARD // NCH           # 512
W2SCALE = 64.0               # host prescale so fp8e4 w2 stays in normal range

PERM = np.zeros(DIM, np.int64)
for _g in range(D4):
    for _c4 in range(4):
        PERM[64 * _c4 + _g] = 4 * _g + _c4
PERM_INV = np.argsort(PERM)


# ---------------------------------------------------------------- host prep
def _pack_inputs(inp):
    x = np.asarray(inp["x"], np.float32)
    xyz = np.asarray(inp["xyz"], np.float32)
    knn = np.asarray(inp["knn"])
    assert knn.dtype == np.int32

    rhs7 = np.zeros((128, DEPTH * 64), np.float32)
    for l in range(DEPTH):
        u = np.asarray(inp["lfp_scale"], np.float32)[l] ** 2
        c = np.asarray(inp["lfp_coor"], np.float32)[l]
        r7 = np.zeros((7, D4), np.float32)
        r7[0:3] = 2.0 * u * c.T
        r7[3:6] = -u
        r7[6] = -u * (c ** 2).sum(-1)
        for rg in range(4):
            rhs7[32 * rg:32 * rg + 7, l * 64:(l + 1) * 64] = r7

    # shared [128, 8] selection block: row p contributes to point p//16, /K
    s8 = np.zeros((128, 8), np.float32)
    for p in range(128):
        s8[p, p // 16] = 1.0 / K

    wproj = np.zeros((128, DEPTH * 2 * DIM), np.float32)
    for l in range(DEPTH):
        w = np.asarray(inp["lfp_proj"], np.float32)[l][PERM][:, PERM]
        for kt in range(2):
            wproj[:, (l * 2 + kt) * DIM:(l * 2 + kt + 1) * DIM] = w[kt * 128:(kt + 1) * 128]

    w1 = np.zeros((128, 3 * 2 * HID), np.float32)
    w2 = np.zeros((128, 3 * 8 * DIM), np.float32)
    b1 = np.zeros((128, 3 * 8), np.float32)
    mg = np.zeros((128, 3 * 2), np.float32)
    mb = np.zeros((128, 3 * 2), np.float32)
    lg = np.zeros((128, DEPTH * 2), np.float32)
    lb = np.zeros((128, DEPTH * 2), np.float32)
    # the device computes s = gamma * rsqrt(var+eps) via a DVE-only
    # bit-trick rsqrt (magic constant + one Newton step), so the BN tail
    # uses no activation-table function at all (gamma > 0 for these inputs).
    for j in range(3):
        a = np.asarray(inp["mlp_w1"], np.float32)[j][PERM]
        for kt in range(2):
            w1[:, (j * 2 + kt) * HID:(j * 2 + kt + 1) * HID] = a[kt * 128:(kt + 1) * 128]
        a = np.asarray(inp["mlp_w2"], np.float32)[j][:, PERM]
        for ht in range(8):
            w2[:, (j * 8 + ht) * DIM:(j * 8 + ht + 1) * DIM] = a[ht * 128:(ht + 1) * 128]
        for ht in range(8):
            b1[:, j * 8 + ht] = np.asarray(inp["mlp_b1"], np.float32)[j][ht * 128:(ht + 1) * 128]
        gj = np.asarray(inp["mlp_gamma"], np.float32)[j][PERM]
        bj = np.asarray(inp["mlp_beta"], np.float32)[j][PERM]
        for ct in range(2):
            mg[:, j * 2 + ct] = gj[ct * 128:(ct + 1) * 128]
            mb[:, j * 2 + ct] = bj[ct * 128:(ct + 1) * 128]
    for l in range(DEPTH):
        gl = np.asarray(inp["lfp_gamma"], np.float32)[l][PERM]
        bl = np.asarray(inp["lfp_beta"], np.float32)[l][PERM]
        for ct in range(2):
            lg[:, l * 2 + ct] = gl[ct * 128:(ct + 1) * 128]
            lb[:, l * 2 + ct] = bl[ct * 128:(ct + 1) * 128]

    shared = {
        "rhs7": rhs7.astype(BF), "ssb": s8.astype(BF), "wproj": wproj.astype(BF),
        "w1": w1.astype(BF), "w2": w2.astype(BF), "b1": b1,
        "mg": mg, "mb": mb, "lg": lg, "lb": lb,
    }

    in_maps = []
    for core in range(NCORES):
        b, sh = core // 4, core % 4
        rows = slice(sh * SHARD, (sh + 1) * SHARD)
        xT0 = np.ascontiguousarray(x[b, rows][:, PERM].T)

        nn = knn[b, rows].reshape(-1).astype(np.int64)          # [32768]
        # wrapped idx layout: per tile t, col t*128+q, partition 16g+p16
        flat = nn.astype(np.int16).reshape(NT, 128, K)          # [t, nl, k]
        flat = flat.reshape(NT, ROWS_T)                         # f = nl*16+k
        idxw = np.zeros((128, NT * 128), np.int16)
        for t in range(NT):
            w = flat[t].reshape(128, 16).T                      # [p16, q]
            for g in range(8):
                idxw[g * 16:(g + 1) * 16, t * 128:(t + 1) * 128] = w

        ctr = np.repeat(np.arange(sh * SHARD, (sh + 1) * SHARD), K)
        pn = (xyz[b, nn] - xyz[b, ctr]).T                       # [3, 32768]
        bas7 = np.concatenate([pn, pn ** 2, np.ones((1, pn.shape[1]), np.float32)], 0)
        basis = np.zeros((128, 16384), np.float32)
        for sg in range(NT * NSLOT):
            rg, cb = sg % 2, sg // 2
            basis[32 * rg:32 * rg + 7, cb * 128:(cb + 1) * 128] = \
                bas7[:, sg * 128:(sg + 1) * 128]

        m = {"xT0": xT0, "xTb0": xT0.astype(BF),
             "idxw": idxw, "basis": basis.astype(BF)}
        m.update(shared)
        in_maps.append(m)
    return in_maps


# ------------------------------------------------------------- device build
def build_program(reps=1, mode="full", skip=()):
    import concourse.bass as bass
    import concourse.bacc as bacc
    import concourse.mybir as mybir
    import concourse.tile as tile
    from concourse import library_config

    f32, bf16, i16 = mybir.dt.float32, mybir.dt.bfloat16, mybir.dt.int16
    fp8 = mybir.dt.float8e4
    AF = mybir.ActivationFunctionType
    OP = mybir.AluOpType
    DR = mybir.MatmulPerfMode.DoubleRow

    noc = mode.endswith("_noag") or mode.endswith("_noc")
    nc = bacc.Bacc("TRN2", target_bir_lowering=False, debug=False,
                   num_devices=NCORES)

    ins = {
        "xT0": nc.dram_tensor("xT0", [DIM, SHARD], f32, kind="ExternalInput").ap(),
        "xTb0": nc.dram_tensor("xTb0", [DIM, SHARD], bf16, kind="ExternalInput").ap(),
        "idxw": nc.dram_tensor("idxw", [128, NT * 128], i16, kind="ExternalInput").ap(),
        "basis": nc.dram_tensor("basis", [128, 16384], bf16, kind="ExternalInput").ap(),
        "rhs7": nc.dram_tensor("rhs7", [128, DEPTH * 64], bf16, kind="ExternalInput").ap(),
        "ssb": nc.dram_tensor("ssb", [128, 8], bf16, kind="ExternalInput").ap(),
        "wproj": nc.dram_tensor("wproj", [128, DEPTH * 2 * DIM], bf16, kind="ExternalInput").ap(),
        "w1": nc.dram_tensor("w1", [128, 3 * 2 * HID], bf16, kind="ExternalInput").ap(),
        "w2": nc.dram_tensor("w2", [128, 3 * 8 * DIM], bf16, kind="ExternalInput").ap(),
        "b1": nc.dram_tensor("b1", [128, 3 * 8], f32, kind="ExternalInput").ap(),
        "mg": nc.dram_tensor("mg", [128, 3 * 2], f32, kind="ExternalInput").ap(),
        "mb": nc.dram_tensor("mb", [128, 3 * 2], f32, kind="ExternalInput").ap(),
        "lg": nc.dram_tensor("lg", [128, DEPTH * 2], f32, kind="ExternalInput").ap(),
        "lb": nc.dram_tensor("lb", [128, DEPTH * 2], f32, kind="ExternalInput").ap(),
    }
    xout = nc.dram_tensor("xout", [DIM, SHARD], bf16, kind="ExternalOutput").ap()
    dbg = {}
    if mode == "debug":
        for nm in ("d_mlp0", "d_lfp0", "d_lfp1"):
            dbg[nm] = nc.dram_tensor(nm, [DIM, SHARD], f32, kind="ExternalOutput").ap()
        dbg["d_aggT"] = nc.dram_tensor("d_aggT", [128, NT * 2 * 128], bf16, kind="ExternalOutput").ap()
        dbg["d_lsum"] = nc.dram_tensor("d_lsum", [128, 2 * NT], f32, kind="ExternalOutput").ap()
        dbg["d_lsq"] = nc.dram_tensor("d_lsq", [128, 2 * NT], f32, kind="ExternalOutput").ap()

    with tile.TileContext(nc) as tc:
        nc.gpsimd.load_library(library_config.mlp)
        with tc.tile_pool(name="const", bufs=1) as cpool, \
             tc.tile_pool(name="state", bufs=1) as spool, \
             tc.tile_pool(name="stage", bufs=1) as stpool, \
             tc.tile_pool(name="deep", bufs=3) as dppool, \
             tc.tile_pool(name="psum", bufs=1, space="PSUM") as pspool, \
             tc.tile_pool(name="dram", bufs=2, space="DRAM") as dpool, \
             tc.tile_pool(name="sdram", bufs=4, space="DRAM") as sdpool:

            # ---- state: bf16 residual (the final f32 conversion happens
            # on the host after the bf16 xout DMA)
            xTb = spool.tile([128, 2, SHARD], bf16, tag="xTb")

            # ---- constants in SBUF; DMA order = consumption order
            c_idx = cpool.tile([128, NT * 128], i16, tag="idx")
            c_bas = cpool.tile([128, 16384], bf16, tag="bas")
            c_r7 = cpool.tile([128, DEPTH * 64], bf16, tag="r7")
            c_s8 = cpool.tile([128, 8], bf16, tag="s")
            c_wp = cpool.tile([128, DEPTH * 2 * DIM], bf16, tag="wp")
            c_w1 = cpool.tile([128, 3 * 2 * HID], bf16, tag="w1")
            c_w2 = cpool.tile([128, 3 * 8 * DIM], bf16, tag="w2")
            c_b1 = cpool.tile([128, 3 * 8], f32, tag="b1")
            c_mg = cpool.tile([128, 3 * 2], f32, tag="mg")
            c_mb = cpool.tile([128, 3 * 2], f32, tag="mb")
            c_lg = cpool.tile([128, DEPTH * 2], f32, tag="lg")
            c_lb = cpool.tile([128, DEPTH * 2], f32, tag="lb")
            for rep in range(reps):
                if rep == 0:
                    # w1/w2 split per-MLP so mlp0 is gated only by its own
                    # j=0 slices; j=1,2 stream in last (needed 100us+ later)
                    W1S, W2S = 2 * HID, 8 * DIM
                    nc.sync.dma_start(c_w1[:, 0:W1S], ins["w1"][:, 0:W1S])
                    nc.sync.dma_start(c_b1[:], ins["b1"][:])
                    sjunk0 = spool.tile([128, 1], f32, tag="sjunk")
                    nc.vector.memset(sjunk0[:], 1.0)
                    nc.scalar.activation(sjunk0[:], sjunk0[:],
                                         mybir.ActivationFunctionType.Gelu_apprx_tanh)
                for q_ in range(NCH):
                    xcs = slice(q_ * CHN, (q_ + 1) * CHN)
                    for ct in range(2):
                        nc.sync.dma_start(
                            xTb[:, ct, xcs],
                            ins["xTb0"][ct * 128:(ct + 1) * 128, xcs])
                if rep == 0:
                    nc.sync.dma_start(c_w2[:, 0:W2S], ins["w2"][:, 0:W2S])
                    # lfp0's proj/gather consts come BEFORE the 11.6us basis
                    # stream so the first table write isn't gated on it
                    for t_, name in ((c_mg, "mg"), (c_mb, "mb"), (c_idx, "idxw"),
                                     (c_r7, "rhs7"), (c_s8, "ssb"),
                                     (c_wp, "wproj"), (c_lg, "lg"),
                                     (c_lb, "lb"), (c_bas, "basis")):
                        nc.sync.dma_start(t_[:], ins[name][:])
                    for j_ in (1, 2):
                        nc.sync.dma_start(c_w1[:, j_ * W1S:(j_ + 1) * W1S],
                                          ins["w1"][:, j_ * W1S:(j_ + 1) * W1S])
                        nc.sync.dma_start(c_w2[:, j_ * W2S:(j_ + 1) * W2S],
                                          ins["w2"][:, j_ * W2S:(j_ + 1) * W2S])

                def allreduce_st(st):
                    """st: [128, 4] f32 tile (sum ct0, sum ct1, sq ct0, sq ct1)
                    of per-core stats; AllReduces across cores, returns stg."""
                    stg = spool.tile([128, 4], f32, tag="stglob")
                    if noc:
                        # debug: local stats scaled up as a stand-in
                        nc.vector.tensor_scalar_mul(stg[:], st[:], float(NCORES))
                        return stg
                    d_in = sdpool.tile([128, 4], f32, tag="st_in")
                    d_out = sdpool.tile([128, 4], f32, tag="st_out")
                    nc.sync.dma_start(d_in[:], st[:])
                    nc.gpsimd.collective_compute(
                        "AllReduce", OP.add,
                        ins=[d_in.opt()], outs=[d_out.opt()],
                        replica_groups=[list(range(NCORES))],
                    )
                    nc.sync.dma_start(stg[:], d_out[:])
                    return stg

                def bn_stats_allreduce(sum3_src, sq3_src):
                    """sum3_src/sq3_src: [128, 2, n] f32 APs of per-chunk
                    partials. Reduces over n, AllReduces across cores, returns
                    stg [128, 4] (sums | sumsqs) of global stats."""
                    st = spool.tile([128, 4], f32, tag="stpack")
                    nc.vector.tensor_reduce(st[:, 0:2], sum3_src,
                                            mybir.AxisListType.X, OP.add)
                    nc.vector.tensor_reduce(st[:, 2:4], sq3_src,
                                            mybir.AxisListType.X, OP.add)
                    return allreduce_st(st)

                sjunk = spool.tile([128, 1], f32, tag="sjunk")

                def hoist_table(func, anchor=None):
                    """Dummy ACT op so the table-set load for `func` lands
                    here (an ACT-idle window) instead of on a critical tail.
                    `anchor` is a [128,1] AP produced at the intended queue
                    position: without a data dependency the tile scheduler
                    floats the (otherwise dep-free) dummy arbitrarily early,
                    scattering table reloads through the steady-state loops."""
                    nc.scalar.activation(sjunk[:], anchor if anchor is not None
                                         else sjunk[:], func)

                magicT = spool.tile([128, 2], mybir.dt.uint32, tag="bn_magic")
                # magic biased by -0x400000: estimates rsqrt(2w) from bits(w)
                nc.vector._memset_packed(magicT[:], 0x5EF759DF)

                def bn_finalize(stg, gam_ap, bet_ap):
                    """s = gamma * rsqrt(var+eps), t = beta - s*mu — all on
                    DVE: bit-trick rsqrt (0x5f3759df) plus one Newton step
                    (max rel err ~1.7e-3), so the BN tail touches no
                    activation table and the Gelu/Exp sets never swap
                    outside phase transitions."""
                    mu = spool.tile([128, 2], f32, tag="bn_mu")
                    var = spool.tile([128, 2], f32, tag="bn_var")
                    varh = spool.tile([128, 2], f32, tag="bn_varh")
                    sfac = spool.tile([128, 2], f32, tag="bn_s")
                    tfac = spool.tile([128, 2], f32, tag="bn_t")
                    y = spool.tile([128, 2], f32, tag="bn_y")
                    t1 = spool.tile([128, 2], f32, tag="bn_t1")
                    nc.vector.tensor_scalar_mul(mu[:], stg[:, 0:2], 1.0 / (B * N))
                    # w = (var+EPS)/2 = (0.5*sumsq/BN + 0.5*EPS) - 0.5*mu^2;
                    # the halving is what the Newton step needs anyway
                    nc.vector.tensor_scalar(var[:], stg[:, 2:4], 0.5 / (B * N),
                                            0.5 * EPS, OP.mult, OP.add)
                    sq = spool.tile([128, 2], f32, tag="bn_sq")
                    nc.vector.scalar_tensor_tensor(sq[:], mu[:], 0.5, mu[:],
                                                   OP.mult, OP.mult)
                    nc.vector.tensor_tensor(varh[:], var[:], sq[:], OP.subtract)
                    # y0 = bitcast(magic' - (bitcast(w) >> 1)) ~ rsqrt(2w)
                    yu = y[:].bitcast(mybir.dt.uint32)
                    nc.vector.tensor_scalar(yu, varh[:].bitcast(mybir.dt.uint32),
                                            1, None, OP.logical_shift_right)
                    nc.vector.tensor_tensor(yu, magicT[:], yu, OP.subtract)
                    # one Newton step toward rsqrt(2w): y = y*(1.5 - w*y^2)
                    nc.vector.tensor_tensor(t1[:], y[:], y[:], OP.mult)
                    nc.vector.tensor_tensor(t1[:], varh[:], t1[:], OP.mult)
                    nc.vector.tensor_scalar(t1[:], t1[:], -1.0, 1.5,
                                            OP.mult, OP.add)
                    nc.vector.tensor_tensor(y[:], y[:], t1[:], OP.mult)
                    nc.vector.tensor_tensor(sfac[:], gam_ap, y[:], OP.mult)
                    nc.vector.tensor_tensor(tfac[:], sfac[:], mu[:], OP.mult)
                    nc.vector.tensor_tensor(tfac[:], bet_ap, tfac[:], OP.subtract)
                    return sfac, tfac

                def apply_update(src_view, sfac, tfac):
                    """xTb += s*src + t, all on DVE: tensor_scalar (4x mode)
                    computes the per-channel affine, then a bf16 2x add.
                    Emitted q-outer (both ct per 1024-col chunk) so the next
                    phase's matmuls start early."""
                    CH = 1024
                    for q in range(SHARD // CH):
                        for ct in range(2):
                            sv = src_view(ct)
                            cs = slice(q * CH, (q + 1) * CH)
                            upd = stpool.tile([128, CH], bf16, tag="upd", bufs=3)
                            if sv.ndim == 3:
                                svq = sv[:, 8 * q:8 * (q + 1), :]
                                uv = upd[:].rearrange("p (a j) -> p a j", j=128)
                            else:
                                svq = sv[:, cs]
                                uv = upd[:]
                            nc.vector.tensor_scalar(
                                uv, svq, sfac[:, ct:ct + 1], tfac[:, ct:ct + 1],
                                OP.mult, OP.add)
                            nc.vector.tensor_tensor(
                                xTb[:, ct, cs], xTb[:, ct, cs], upd[:], OP.add)

                def mlp(j, next_func=None):
                    h1b = stpool.tile([128, 8, CHN], bf16, tag="h1b", bufs=2)
                    h2b = stpool.tile([128, 2, SHARD], bf16, tag="h2b")
                    junk = stpool.tile([128, CHN], bf16, tag="junk")
                    sums = stpool.tile([128, 2, NCH], f32, tag="msum")
                    sqs = stpool.tile([128, 2, NCH], f32, tag="msq")
                    for nch in range(NCH):
                        n0 = nch * CHN
                        for ht in range(8):
                            p1 = pspool.tile([128, CHN], mybir.dt.float32, tag="pa", bufs=4)
                            for kt in range(2):
                                nc.tensor.matmul(
                                    p1[:],
                                    c_w1[:, (j * 2 + kt) * HID + ht * 128:
                                         (j * 2 + kt) * HID + (ht + 1) * 128],
                                    xTb[:, kt, n0:n0 + CHN],
                                    start=(kt == 0), stop=(kt == 1))
                            nc.scalar.activation(h1b[:, ht, :], p1[:],
                                                 AF.Gelu_apprx_tanh,
                                                 bias=c_b1[:, j * 8 + ht:j * 8 + ht + 1])
                        for ct in range(2):
                            p2 = pspool.tile([128, CHN], mybir.dt.float32, tag="pb", bufs=2)
                            for ht in range(8):
                                nc.tensor.matmul(
                                    p2[:],
                                    c_w2[:, (j * 8 + ht) * DIM + ct * 128:
                                         (j * 8 + ht) * DIM + (ct + 1) * 128],
                                    h1b[:, ht, :],
                                    start=(ht == 0), stop=(ht == 7))
                            # psum drain + BN channel-sums in one DVE pass
                            nc.vector.tensor_scalar(
                                h2b[:, ct, n0:n0 + CHN], p2[:],
                                1.0, 0.0, OP.mult, OP.add,
                                accum_out=sums[:, ct, nch:nch + 1])
                            nc.vector.scalar_tensor_tensor(
                                junk[:], h2b[:, ct, n0:n0 + CHN], 1.0,
                                h2b[:, ct, n0:n0 + CHN], OP.mult, OP.mult,
                                accum_out=sqs[:, ct, nch:nch + 1])
                    stg = bn_stats_allreduce(sums[:], sqs[:])
                    sfac, tfac = bn_finalize(stg, c_mg[:, j * 2:j * 2 + 2],
                                             c_mb[:, j * 2:j * 2 + 2])
                    if next_func is not None:
                        hoist_table(next_func, tfac[:, 0:1])
                    apply_update(lambda ct: h2b[:, ct, :], sfac, tfac)

                def lfp(l, next_func=None):
                    hsh = stpool.tile([128, NT, DIM], bf16, tag="hsh")
                    bounce = dpool.tile([SHARD, DIM], bf16, tag="bounce")
                    table = dpool.tile([N, DIM], bf16, tag="table")
                    bview = bounce.rearrange("(t p) c -> p t c", p=128)

                    def d2exp(t):
                        """emit d2 matmuls + Exp for tile t; returns wgt tile.
                        2-way row-group concurrency: each concurrent group must
                        write a distinct PSUM bank (same-bank concurrent PE
                        writes fault the exec unit). slot s -> bank s%2, 64-col
                        sub-offset s//2; positions 0/32 alternate so at most
                        two matmuls overlap, in distinct banks. bufs=2 so the
                        exp(t) read does not serialize against mm(t+1) — a
                        single buffer queues the late exps behind the
                        critical tail drains on ACT."""
                        pd2 = pspool.tile([128, 2, 512], mybir.dt.float32,
                                          tag="pd2", bufs=1)
                        for s in range(NSLOT):
                            sg = t * NSLOT + s
                            rg, cb = sg % 2, sg // 2
                            nc.tensor.matmul(
                                pd2[:, s % 2, (s // 2) * 64:(s // 2 + 1) * 64],
                                c_bas[32 * rg:32 * rg + 7, cb * 128:(cb + 1) * 128],
                                c_r7[32 * rg:32 * rg + 7, l * 64:(l + 1) * 64],
                                start=True, stop=True,
                                tile_position=(32 * rg, 0))
                        wgt = dppool.tile([128, NSLOT * 64], bf16, tag="wgt", bufs=5)
                        # wgt col (q*2+s2)*64+g <- pd2[:, s2, q*64+g]
                        nc.scalar.activation(
                            wgt[:].rearrange("p (q s2 g) -> p s2 q g", s2=2, g=64),
                            pd2[:, :, 0:512].rearrange("p s2 (q g) -> p s2 q g", g=64),
                            AF.Exp)
                        return wgt

                    # 1) proj h-shard row-major (chunked bounce write), and
                    #    wgt for tile 0 ahead of the pipeline; the prefetched
                    #    d2exps are priority-deboosted so their Exps don't
                    #    schedule ahead of the previous layer's agg drains
                    with tc.high_priority(offset=-(1 << 20)):
                        wgt_next = d2exp(0)
                    for t in range(NT):
                        ph = pspool.tile([128, DIM], mybir.dt.float32, tag="pa", bufs=4)
                        for kt in range(2):
                            nc.tensor.matmul(
                                ph[:],
                                xTb[:, kt, t * 128:(t + 1) * 128],
                                c_wp[:, (l * 2 + kt) * DIM:(l * 2 + kt + 1) * DIM],
                                start=(kt == 0), stop=(kt == 1))
                        # early drains on ACT (DVE still finishing the BN
                        # apply), late ones on DVE once the apply is done
                        if t < 8:
                            nc.scalar.activation(hsh[:, t, :], ph[:], AF.Copy)
                        else:
                            nc.vector.tensor_copy(hsh[:, t, :], ph[:])
                        if t % 4 == 3 and not noc:
                            q = t // 4
                            nc.sync.dma_start(bview[:, 4 * q:4 * (q + 1), :],
                                              hsh[:, 4 * q:4 * (q + 1), :])
                    if noc:
                        # stand-in: own shard only (models the local table
                        # write), sourced straight from SBUF to shorten the
                        # proj->table->gather critical chain. Two half-shard
                        # writes: one dma_start per half avoids the ~1.6us
                        # per-instruction SP/HWDGE issue pipeline of the old
                        # 4-chunk emission while still overlapping the first
                        # half with the second half's proj drains.
                        tview = table.rearrange("(t p) c -> p t c", p=128)
                        for q in range(4):
                            nc.sync.dma_start(tview[:, 4 * q:4 * (q + 1), :],
                                              hsh[:, 4 * q:4 * (q + 1), :])
                        for q in range(4):
                            nc.sync.dma_start(bview[:, 4 * q:4 * (q + 1), :],
                                              hsh[:, 4 * q:4 * (q + 1), :])
                    else:
                        nc.gpsimd.collective_compute(
                            "AllGather", OP.bypass,
                            ins=[bounce.opt()], outs=[table.opt()],
                            replica_groups=[[0, 1, 2, 3], [4, 5, 6, 7]],
                        )

                    # 2) pipelined per-tile: gather -> mult (in-place) ->
                    #    select -> fused drain -> DVE stats; d2/exp runs three
                    #    tiles ahead. The select is 16 disjoint 8-col matmuls
                    #    per ct (rows p of a slot hit point p//16), nearly
                    #    free on PE; BN sums ride a 4x-mode tensor_scalar.
                    aggT = stpool.tile([128, NT, 2, 128], bf16, tag="aggT")
                    lsum = stpool.tile([128, 2, NT], f32, tag="lsum")
                    lsq = stpool.tile([128, 2, NT], f32, tag="lsq")
                    ljunk = stpool.tile([128, 128], bf16, tag="ljunk")
                    ljunk2 = stpool.tile([128, 128], bf16, tag="ljunk2")

                    def select(t, hn):
                        """k-reduction via disjoint 8-col selection matmuls
                        (slot s feeds points s*8..s*8+8 exclusively), fused
                        two-ct PSUM drain on ACT, per-tile BN stats on DVE."""
                        pag = pspool.tile([128, 2, 128], mybir.dt.float32, tag="pb", bufs=2)
                        for ct in range(2):
                            for s in range(NSLOT):
                                nc.tensor.matmul(
                                    pag[:, ct, s * 8:(s + 1) * 8],
                                    hn[:, s, ct * 128:(ct + 1) * 128],
                                    c_s8[:, 0:8],
                                    start=True, stop=True)
                        nc.scalar.activation(aggT[:, t, :, :], pag[:], AF.Copy)
                        for ct in range(2):
                            nc.vector.tensor_scalar(
                                ljunk[:], aggT[:, t, ct, :], 1.0, 0.0,
                                OP.mult, OP.add,
                                accum_out=lsum[:, ct, t:t + 1])
                            nc.vector.scalar_tensor_tensor(
                                ljunk2[:], aggT[:, t, ct, :], 1.0,
                                aggT[:, t, ct, :], OP.mult, OP.mult,
                                accum_out=lsq[:, ct, t:t + 1])

                    LEAD, SLAG = 3, 2
                    wgts = {0: wgt_next}
                    with tc.high_priority(offset=-(1 << 20)):
                        for t in range(1, LEAD):
                            wgts[t] = d2exp(t)
                    hns = {}
                    for t in range(NT):
                        hn = dppool.tile([128, NSLOT, DIM], bf16,
                                         tag="hn", bufs=5)
                        hns[t] = hn
                        # last tile: two half-gathers (idx cols map linearly
                        # to row indices: col q holds rows q*16+k) so the
                        # drain's mult can start on slots 0-7 a half-gather
                        # earlier
                        nhalf = 2 if t == NT - 1 else 1
                        for h_ in range(nhalf):
                            hs = NSLOT // nhalf
                            nc.gpsimd.dma_gather(
                                out_ap=hn[:, h_ * hs:(h_ + 1) * hs, :],
                                in_ap=table[:],
                                idxs_ap=c_idx[:, t * 128 + h_ * (128 // nhalf):
                                              t * 128 + (h_ + 1) * (128 // nhalf)],
                                num_idxs=ROWS_T // nhalf,
                                num_idxs_reg=ROWS_T // nhalf,
                                elem_size=DIM,
                                single_packet=False,
                            )
                        if t + LEAD < NT:
                            wgts[t + LEAD] = d2exp(t + LEAD)
                        wgt = wgts.pop(t)
                        hn4 = hn[:].rearrange("p s (c4 g) -> p s c4 g", g=64)
                        wgt_b = (wgt[:].rearrange("p (s g) -> p s g", g=64)
                                 .unsqueeze(2).broadcast_to([128, NSLOT, 4, 64]))
                        # last tile: the mult runs per half-gather so the
                        # drain chain starts a half-gather earlier
                        for _m in range(nhalf):
                            hm = NSLOT // nhalf
                            ms = slice(_m * hm, (_m + 1) * hm)
                            nc.vector.tensor_tensor(hn4[:, ms], hn4[:, ms],
                                                    wgt_b[:, ms], OP.mult)
                        if t >= SLAG:
                            select(t - SLAG, hns.pop(t - SLAG))
                    for t in range(NT - SLAG, NT):
                        select(t, hns.pop(t))

                    if mode == "debug" and l == 0:
                        nc.sync.dma_start(dbg["d_aggT"].rearrange(
                            "p (t ct j) -> p t ct j", t=NT, ct=2), aggT[:])
                        nc.sync.dma_start(dbg["d_lsum"].rearrange(
                            "p (ct t) -> p ct t", ct=2), lsum[:])
                        nc.sync.dma_start(dbg["d_lsq"].rearrange(
                            "p (ct t) -> p ct t", ct=2), lsq[:])

                    # 3) stats reduce + BN + residual
                    stg = bn_stats_allreduce(lsum[:], lsq[:])
                    sfac, tfac = bn_finalize(stg, c_lg[:, l * 2:l * 2 + 2],
                                             c_lb[:, l * 2:l * 2 + 2])
                    if next_func is not None:
                        hoist_table(next_func, tfac[:, 0:1])
                    apply_update(lambda ct: aggT[:, :, ct, :], sfac, tfac)

                if mode == "debug":
                    xdbg = spool.tile([128, 2, SHARD], f32, tag="xdbg")

                    def dump(nm):
                        for ct in range(2):
                            nc.vector.tensor_copy(xdbg[:, ct, :], xTb[:, ct, :])
                        nc.sync.dma_start(
                            dbg[nm].rearrange("(c p) n -> p c n", p=128), xdbg[:])
                    mlp(0); dump("d_mlp0")
                    lfp(0); dump("d_lfp0")
                    lfp(1); dump("d_lfp1")
                do_mlp = mode in ("full", "mlp0", "full_noc")
                do_lfp = mode in ("full", "lfp0", "lfp0_noag", "full_noc")
                n_lfp = DEPTH if mode in ("full", "full_noc") else (1 if do_lfp else 0)
                base_mode = mode.replace("_noag", "").replace("_noc", "")
                if base_mode == "mlp2":
                    mlp(0); mlp(1)
                elif base_mode == "lfp2":
                    lfp(0); lfp(1)
                elif base_mode == "mlp3":
                    mlp(0); mlp(1); mlp(2)
                elif base_mode == "lfp3":
                    lfp(0); lfp(1); lfp(2)
                else:
                    full = mode in ("full", "full_noc")
                    if do_mlp:
                        mlp(0, next_func=AF.Exp if (full or do_lfp) else None)
                    for l in range(n_lfp):
                        nxt = None
                        if full:
                            nxt = (AF.Gelu_apprx_tanh if l % 2 == 1
                                   else AF.Exp)
                        lfp(l, next_func=nxt)
                        if l % 2 == 1 and full:
                            mlp(1 + l // 2,
                                next_func=AF.Exp if l + 1 < n_lfp else None)
            # residual state is bf16 end-to-end: ship it out as-is (the
            # host upcasts) -- the old f32 copy pass added no precision
            xov = xout.rearrange("(c p) n -> p c n", p=128)
            for q in range(4):
                cs = slice(q * 512, (q + 1) * 512)
                nc.sync.dma_start(xov[:, :, cs], xTb[:, :, cs])

    nc.compile()
    return nc


_NC_CACHE = {}


def _get_nc(reps=1, mode="full"):
    key = (reps, mode)
    if key not in _NC_CACHE:
        _NC_CACHE[key] = build_program(reps, mode)
    return _NC_CACHE[key]


def run_on_cores(in_maps, reps=1, mode="full"):
    from concourse.bass_utils import run_bass_kernel_spmd
    nc = _get_nc(reps, mode)
    return run_bass_kernel_spmd(nc, in_maps, core_ids=list(range(NCORES)))


def kernel(**inputs):
    in_maps = _pack_inputs(inputs)
    res = None
    for attempt in range(4):
        try:
            res = run_on_cores(in_maps, reps=1)
            break
        except Exception:
            # transient device-state faults occur on this fleet; back off and
            # retry on a fresh dispatch (observed to clear them)
            if attempt == 3:
                raise
            import time as _time
            _time.sleep(5.0)
            try:
                import jax
                jax.clear_caches()
            except Exception:
                pass
    out = np.zeros((B, N, DIM), np.float32)
    for core in range(NCORES):
        b, sh = core // 4, core % 4
        out[b, sh * SHARD:(sh + 1) * SHARD] = \
            res.results[core]["xout"].astype(np.float32).T[:, PERM_INV]
    return out.astype(np.float32)
